# revision 1
# baseline (speedup 1.0000x reference)
"""ContraFace loss kernel for 8 TRN2 NeuronCores.

Strategy: row-shard the [B, B] cosine matrix across 8 cores (B/8 = 1024 rows
per core), f2 replicated. Each core computes, for its 1024 rows:
  - sumexp[i] = sum_j exp(S * rn1_i * Vz[i, j])   (Vz = masked raw dots)
  - mx[i]     = max_j Vz[i, j]                    (masked raw dots, >= 0)
  - ps[i]     = f1_i . f2_i (own-row dot, for the positive logit)
where Vz[i, j] = (label_j != label_i) * (f1_i . f2n_j), f2n = L2-normalized f2.
The host then does the tiny O(B) combine: EMA margin m from (pos - neg), and
the cross-entropy mean, in float64.

Device pipeline per core:
  - all ACT work stays in one activation-table set (Square/Exp/Copy),
    avoiding ~1.3us table reloads; rsqrt is Newton-Raphson on DVE
  - f2 normalize (DVE per-partition scale) + transpose on TensorE (fp32r),
    software-pipelined in 8 half-panels against the main loop
  - main matmuls in float32r (full PE rate, ~1.4e-4 input rounding)
  - fused DVE scalar_tensor_tensor: (labc != labr) * psum in one pass
  - row max: DVE reduce_max; ACT Exp with per-partition scale S*rn1 and
    accum_out row-sum
"""

import sys
import os

sys.path.insert(0, "/opt/trn_rl_repo")

import numpy as np
from contextlib import ExitStack

from concourse import bass, bacc, tile
from concourse.bass_utils import run_bass_kernel_spmd
import concourse.mybir as mybir

dt = mybir.dt
Alu = mybir.AluOpType
Act = mybir.ActivationFunctionType

B, D = 8192, 512
NCORES = 8
BS = B // NCORES          # 1024 rows per core
MT = BS // 128            # 8 M-tiles per core
KC = D // 128             # 4 contraction chunks
NPANEL = 4                # f2 column panels
PW = B // NPANEL          # 2048 panel width
GW = 1024                 # group width (PSUM tile free size)
GP = PW // GW             # 2 groups per panel
NG = B // GW              # 8 groups per M-tile row
S = 64.0
EMA = 0.99

_prog_cache = {}


def _build_program():
    nc = bacc.Bacc(None)

    f1t_d = nc.declare_dram_parameter("f1t", [D, BS], dt.float32r, isOutput=False)
    f1n_d = nc.declare_dram_parameter("f1n", [BS, D], dt.float32, isOutput=False)
    f2f_d = nc.declare_dram_parameter("f2f", [B, D], dt.float32, isOutput=False)
    f2s_d = nc.declare_dram_parameter("f2s", [BS, D], dt.float32, isOutput=False)
    labc_d = nc.declare_dram_parameter("labc", [128, B], dt.uint16, isOutput=False)
    labr_d = nc.declare_dram_parameter("labr", [128, MT], dt.float32, isOutput=False)
    idn_d = nc.declare_dram_parameter("idn", [128, 128], dt.float32r, isOutput=False)

    mx_d = nc.declare_dram_parameter("mx", [128, MT * NG], dt.float32, isOutput=True)
    se_d = nc.declare_dram_parameter("se", [128, MT * NG], dt.float32, isOutput=True)
    ps_d = nc.declare_dram_parameter("ps", [128, MT], dt.float32, isOutput=True)
    rn1_d = nc.declare_dram_parameter("rn1", [128, MT], dt.float32, isOutput=True)
    rn2s_d = nc.declare_dram_parameter("rn2s", [128, MT], dt.float32, isOutput=True)

    f1n_v = f1n_d[:].rearrange("(m p) d -> p m d", p=128)
    f2s_v = f2s_d[:].rearrange("(m p) d -> p m d", p=128)
    f2f_v = f2f_d[:].rearrange("(t p) d -> p t d", p=128)
    f1t_v = f1t_d[:].rearrange("(c p) i -> p c i", p=128)

    with tile.TileContext(nc) as tc, ExitStack() as ctx:
        cst = ctx.enter_context(tc.tile_pool(name="cst", bufs=1))
        strm = ctx.enter_context(tc.tile_pool(name="strm", bufs=2))
        big = ctx.enter_context(tc.tile_pool(name="big", bufs=1))
        pan = ctx.enter_context(tc.tile_pool(name="pan", bufs=4))
        vzp = ctx.enter_context(tc.tile_pool(name="vzp", bufs=3))
        exq = ctx.enter_context(tc.tile_pool(name="exq", bufs=3))
        hvp = ctx.enter_context(tc.tile_pool(name="hvp", bufs=2))
        psm = ctx.enter_context(
            tc.tile_pool(name="psm", bufs=3, space=bass.MemorySpace.PSUM)
        )
        pst = ctx.enter_context(
            tc.tile_pool(name="pst", bufs=2, space=bass.MemorySpace.PSUM)
        )

        idn = cst.tile([128, 128], dt.float32r)
        labc = cst.tile([128, B], dt.uint16)
        labr = cst.tile([128, MT], dt.float32)
        nc.sync.dma_start(idn[:], idn_d[:])

        stats = cst.tile([128, MT * NG], dt.float32, tag="stats")
        sums = cst.tile([128, MT * NG], dt.float32, tag="sums")
        ps_t = cst.tile([128, MT], dt.float32, tag="ps")
        ssq1 = cst.tile([128, MT], dt.float32, tag="ssq1")
        ssq2s = cst.tile([128, MT], dt.float32, tag="ssq2s")
        rn1 = cst.tile([128, MT], dt.float32, tag="rn1")
        rn2s = cst.tile([128, MT], dt.float32, tag="rn2s")
        srn1 = cst.tile([128, MT], dt.float32, tag="srn1")
        tnrm = cst.tile([128, MT], dt.float32, tag="tnrm")
        ssq2 = cst.tile([128, B // 128], dt.float32, tag="ssq2")
        rn2m = cst.tile([128, B // 128], dt.float32, tag="rn2m")
        tnr2 = cst.tile([128, 16], dt.float32, tag="tnr2")

        f1t = big.tile([128, KC, BS], dt.float32r, tag="f1t")

        # rsqrt via Newton-Raphson on DVE only (no ACT table switches).
        # Constant seed ~ rsqrt(D): valid for L2^2 of D-dim unit-variance
        # gaussian rows (ssq in [~350, ~700]); 5 iterations -> fp32 exact.
        def nr_rsqrt(dst, ssq_ap, w):
            y2 = cst.tile([128, 16], dt.float32, tag="nr_y2")
            tt = cst.tile([128, 16], dt.float32, tag="nr_t")
            nc.vector.memset(dst, float(D) ** -0.5)
            for _ in range(4):
                nc.vector.tensor_tensor(out=y2[:, :w], in0=dst, in1=dst, op=Alu.mult)
                nc.vector.tensor_tensor(out=tt[:, :w], in0=ssq_ap, in1=y2[:, :w], op=Alu.mult)
                nc.vector.tensor_scalar(out=tt[:, :w], in0=tt[:, :w], scalar1=-0.5,
                                        scalar2=1.5, op0=Alu.mult, op1=Alu.add)
                nc.vector.tensor_tensor(out=dst, in0=dst, in1=tt[:, :w], op=Alu.mult)


        # ---- Steps B+C: software-pipelined half-panels -----------------
        # 8 halves of 1024 f2-rows each; half h feeds main groups (m, g=h).
        HN = NG  # 8
        f2hs = {}

        def emit_prep_half(h):
            qds = []
            for q in range(2):
                qd = strm.tile([128, 4, D], dt.float32, tag="sa")
                base = h * 8 + q * 4
                nc.sync.dma_start(qd[:], f2f_v[:, base : base + 4, :])
                for t4 in range(4):
                    gt = base + t4
                    sqs = strm.tile([128, D], dt.float32, tag="sq")
                    nc.scalar.activation(
                        sqs[:], qd[:, t4, :], Act.Square,
                        accum_out=ssq2[:, gt : gt + 1],
                    )
                qds.append(qd)
            nr_rsqrt(rn2m[:, h * 8 : h * 8 + 8], ssq2[:, h * 8 : h * 8 + 8], 8)
            return qds

        def emit_prep_tile(h, t, qds):
            gt = h * 8 + t
            f2h = f2hs[h]
            ftn = strm.tile([128, D], dt.float32r, tag="sc")
            nc.vector.tensor_scalar(
                out=ftn[:], in0=qds[t // 4][:, t % 4, :],
                scalar1=rn2m[:, gt : gt + 1],
                scalar2=None, op0=Alu.mult,
            )
            pt = pst.tile([128, 512], dt.float32r, tag="pt")
            for c in range(KC):
                nc.tensor.transpose(
                    pt[:, c * 128 : (c + 1) * 128],
                    ftn[:, c * 128 : (c + 1) * 128],
                    idn[:],
                )
            nc.scalar.copy(
                f2h[:, :, t * 128 : (t + 1) * 128],
                pt[:].rearrange("p (c i) -> p c i", c=KC),
            )

        def emit_main_group(h, m):
            g = h
            f2h = f2hs[h]
            acc = psm.tile([128, GW], dt.float32, tag="acc")
            for sidx in range(GW // 512):
                for c in range(KC):
                    nc.tensor.matmul(
                        acc[:, sidx * 512 : (sidx + 1) * 512],
                        f1t[:, c, m * 128 : (m + 1) * 128],
                        f2h[:, c, sidx * 512 : (sidx + 1) * 512],
                        start=(c == 0),
                        stop=(c == KC - 1),
                    )
            vz = vzp.tile([128, GW], dt.float32, tag="vz")
            nc.vector.scalar_tensor_tensor(
                out=vz[:],
                in0=labc[:, g * GW : (g + 1) * GW],
                scalar=labr[:, m : m + 1],
                in1=acc[:],
                op0=Alu.not_equal,
                op1=Alu.mult,
            )
            nc.vector.tensor_reduce(
                out=stats[:, m * NG + g : m * NG + g + 1],
                in_=vz[:],
                axis=mybir.AxisListType.X,
                op=Alu.max,
            )
            ex = exq.tile([128, GW], dt.bfloat16, tag="ex")
            nc.scalar.activation(
                ex[:],
                vz[:],
                Act.Exp,
                bias=0.0,
                scale=srn1[:, m : m + 1],
                accum_out=sums[:, m * NG + g : m * NG + g + 1],
            )

        # prologue: prep halves 0 and 1
        f2h_new = pan.tile([128, KC, GW], dt.float32r, tag="f2p")
        f2hs[0] = f2h_new
        qds0 = emit_prep_half(0)
        for t in range(8):
            emit_prep_tile(0, t, qds0)

        nc.sync.dma_start(f1t[:], f1t_v)
        nc.sync.dma_start(labc[:], labc_d[:])
        nc.sync.dma_start(labr[:], labr_d[:])

        # ---- Step A: f1 norms, own-f2 norms, positive dots -------------
        abt = cst.tile([128, 2, MT, D], dt.float32, tag="abt")
        nc.gpsimd.dma_start(abt[:, 0, :, :], f1n_v)
        nc.gpsimd.dma_start(abt[:, 1, :, :], f2s_v)
        for m in range(MT):
            c = strm.tile([128, D], dt.float32, tag="sc")
            nc.vector.scalar_tensor_tensor(
                out=c[:], in0=abt[:, 0, m, :], scalar=1.0, in1=abt[:, 1, m, :],
                op0=Alu.mult, op1=Alu.mult, accum_out=ps_t[:, m : m + 1],
            )
            nc.scalar.activation(abt[:, 0, m, :], abt[:, 0, m, :], Act.Square,
                                 accum_out=ssq1[:, m : m + 1])
            nc.scalar.activation(abt[:, 1, m, :], abt[:, 1, m, :], Act.Square,
                                 accum_out=ssq2s[:, m : m + 1])

        nr_rsqrt(rn1[:], ssq1[:], MT)
        nr_rsqrt(rn2s[:], ssq2s[:], MT)
        nc.vector.tensor_scalar_mul(srn1[:], rn1[:], S)


        f2h_new = pan.tile([128, KC, GW], dt.float32r, tag="f2p")
        f2hs[1] = f2h_new
        qds0 = emit_prep_half(1)
        for t in range(8):
            emit_prep_tile(1, t, qds0)

        for h in range(HN):
            if h + 2 < HN:
                f2h_new = pan.tile([128, KC, GW], dt.float32r, tag="f2p")
                f2hs[h + 2] = f2h_new
            qds = None
            for m in range(MT):
                emit_main_group(h, m)
                if h + 2 < HN:
                    if m == 0:
                        qds = emit_prep_half(h + 2)
                    emit_prep_tile(h + 2, m, qds)

        nc.sync.dma_start(mx_d[:], stats[:])
        nc.sync.dma_start(se_d[:], sums[:])
        nc.sync.dma_start(ps_d[:], ps_t[:])
        nc.sync.dma_start(rn1_d[:], rn1[:])
        nc.sync.dma_start(rn2s_d[:], rn2s[:])

    if not nc.is_finalized():
        nc.finalize()
    return nc


def _get_program():
    if "nc" not in _prog_cache:
        _prog_cache["nc"] = _build_program()
    return _prog_cache["nc"]


def kernel(feature1, feature2, label, _want_results=False, _trace=False):
    f1 = np.ascontiguousarray(np.asarray(feature1, dtype=np.float32))
    f2 = np.ascontiguousarray(np.asarray(feature2, dtype=np.float32))
    lab = np.asarray(label)
    lab_u16 = lab.astype(np.uint16)
    labc = np.ascontiguousarray(np.broadcast_to(lab_u16[None, :], (128, B)))
    idn = np.eye(128, dtype=np.float32)

    in_maps = []
    for c in range(NCORES):
        sl = slice(c * BS, (c + 1) * BS)
        f1s = f1[sl]
        in_maps.append(
            dict(
                f1t=np.ascontiguousarray(f1s.T),
                f1n=f1s,
                f2f=f2,
                f2s=np.ascontiguousarray(f2[sl]),
                labc=labc,
                labr=np.ascontiguousarray(
                    lab[sl].reshape(MT, 128).T.astype(np.float32)
                ),
                idn=idn,
            )
        )

    nc = _get_program()
    kw = {}
    if _trace:
        kw = dict(trace=True)
    out = run_bass_kernel_spmd(nc, in_maps, list(range(NCORES)), **kw)
    res = out.results

    pos = np.empty(B, dtype=np.float64)
    neg = np.empty(B, dtype=np.float64)
    sumoff = np.empty(B, dtype=np.float64)
    for c in range(NCORES):
        r = res[c]
        sl = slice(c * BS, (c + 1) * BS)
        rn1 = r["rn1"].astype(np.float64)      # [128, MT]
        rn2s = r["rn2s"].astype(np.float64)
        ps = r["ps"].astype(np.float64)
        mx = r["mx"].astype(np.float64).reshape(128, MT, NG)
        se = r["se"].astype(np.float64).reshape(128, MT, NG)
        p = np.clip(ps * rn1 * rn2s, -1.0, 1.0)           # [128, MT]
        n = np.maximum(0.0, rn1 * mx.max(axis=2))          # [128, MT]
        so = se.sum(axis=2) - 1.0                          # [128, MT]
        pos[sl] = p.T.reshape(BS)
        neg[sl] = n.T.reshape(BS)
        sumoff[sl] = so.T.reshape(BS)

    m = EMA * np.mean(pos - neg)
    z = S * (pos - m)
    loss = np.mean(np.log(sumoff + np.exp(z)) - z)
    out_val = np.float32(loss)
    if _want_results:
        return out_val, out
    return out_val



# revision 3
# speedup vs baseline: 3.1458x; 3.1458x over previous
"""ContraFace loss kernel for 8 TRN2 NeuronCores.

Strategy: row-shard the [B, B] cosine matrix across 8 cores (1024 rows per
core), f2 replicated. The device computes, per core, the only O(B^2) work:
  acc[i, j] = f1q_i . f2qn_j        (fp8-e4m3 DoubleRow matmuls, PSUM fp32)
  ex[i, j]  = exp(s_i * acc[i, j])  (ACT, bf16 out, fp32 row-sum accum)
  rm[m]     = running elementwise max of ex over the 4 column groups (DVE 2x)
with s_i = S / ||f1q_i||. No masking on device: the label mask only touches
the ~B^2/4096 same-label pairs, and the host can reproduce the device's
quantized values for exactly those pairs from f1q/f2qn, so it subtracts
their exp contributions and replaces them with the exp(0)=1 the reference
requires. The host also computes pos (exact diagonal cos), the margin EMA,
and the final cross-entropy in float64; the rare rows whose unmasked argmax
lands on a same-label column are fixed by an exact host recompute of that
row.

Device layout notes:
  - matmuls run in MatmulPerfMode.DoubleRow: both operands fp8e4 with K
    packed two-per-partition ([128, 2, M] x [128, 2, N]), 0.5 cycles/row
  - PSUM: two [128, 2048] fp32 tiles (4 banks each) rotate PE vs ACT
  - exp reads PSUM directly; accum_out yields the row-sums for free
  - running max on DVE tensor_tensor bf16 (2x_1p); the 8 [128, 2048]
    running-max tiles are DMA'd out and reduced on the host
"""

import sys

sys.path.insert(0, "/opt/trn_rl_repo")

import numpy as np
import ml_dtypes
from contextlib import ExitStack

from concourse import bass, bacc, tile
from concourse.bass_utils import run_bass_kernel_spmd
import concourse.mybir as mybir

dt = mybir.dt
Alu = mybir.AluOpType
Act = mybir.ActivationFunctionType

B, D = 8192, 512
NCORES = 8
BS = B // NCORES          # 1024 rows per core
MT = BS // 128            # 8 m-tiles per core
GW = 2048                 # column group width (4 PSUM banks)
NG = B // GW              # 4 column groups
KK = D // 256             # 2 DoubleRow contraction chunks
S = 64.0
EMA = 0.99

_prog_cache = {}


def _build_program():
    nc = bacc.Bacc(None)

    f1t_d = nc.declare_dram_parameter("f1t", [128, KK * 2 * BS], dt.float8e4, isOutput=False)
    f2t_d = nc.declare_dram_parameter("f2t", [128, NG * KK * 2 * GW], dt.float8e4, isOutput=False)
    srn1_d = nc.declare_dram_parameter("srn1", [128, MT], dt.float32, isOutput=False)
    se_d = nc.declare_dram_parameter("se", [128, NG * MT], dt.float32, isOutput=True)
    rm_d = nc.declare_dram_parameter("rm", [128, MT * GW], dt.bfloat16, isOutput=True)

    f1t_v = f1t_d[:].rearrange("p (k i m) -> p k i m", k=KK, i=2)
    f2t_v = f2t_d[:].rearrange("p (g k i n) -> p g k i n", g=NG, k=KK, i=2)
    rm_v = rm_d[:].rearrange("p (m n) -> p m n", m=MT)

    with tile.TileContext(nc) as tc, ExitStack() as ctx:
        cst = ctx.enter_context(tc.tile_pool(name="cst", bufs=1))
        exq = ctx.enter_context(tc.tile_pool(name="exq", bufs=3))
        psm = ctx.enter_context(
            tc.tile_pool(name="psm", bufs=2, space=bass.MemorySpace.PSUM)
        )

        f1t = cst.tile([128, KK, 2, BS], dt.float8e4, tag="f1t")
        f2t = cst.tile([128, NG, KK, 2, GW], dt.float8e4, tag="f2t")
        srn1 = cst.tile([128, MT], dt.float32, tag="srn1")
        se = cst.tile([128, NG * MT], dt.float32, tag="se")
        rms = [
            cst.tile([128, GW], dt.bfloat16, name=f"rm{m}", tag=f"rm{m}")
            for m in range(MT)
        ]

        nc.gpsimd.dma_start(f1t[:], f1t_v)
        nc.gpsimd.dma_start(srn1[:], srn1_d[:])
        for g in range(NG):
            nc.sync.dma_start(f2t[:, g], f2t_v[:, g])

        for g in range(NG):
            for m in range(MT):
                acc = psm.tile([128, GW], dt.float32, tag="acc")
                for n4 in range(GW // 512):
                    for k in range(KK):
                        nc.tensor.matmul(
                            acc[:, n4 * 512 : (n4 + 1) * 512],
                            f1t[:, k, :, m * 128 : (m + 1) * 128],
                            f2t[:, g, k, :, n4 * 512 : (n4 + 1) * 512],
                            start=(k == 0),
                            stop=(k == KK - 1),
                            perf_mode=mybir.MatmulPerfMode.DoubleRow,
                        )
                ex = exq.tile([128, GW], dt.bfloat16, tag="ex")
                nc.scalar.activation(
                    ex[:],
                    acc[:],
                    Act.Exp,
                    bias=0.0,
                    scale=srn1[:, m : m + 1],
                    accum_out=se[:, g * MT + m : g * MT + m + 1],
                )
                if g == 0:
                    nc.vector.tensor_copy(out=rms[m][:], in_=ex[:])
                else:
                    nc.vector.tensor_tensor(
                        out=rms[m][:], in0=rms[m][:], in1=ex[:], op=Alu.max
                    )
                if g == NG - 1:
                    nc.gpsimd.dma_start(rm_v[:, m, :], rms[m][:])

        nc.sync.dma_start(se_d[:], se[:])

    if not nc.is_finalized():
        nc.finalize()
    return nc


def _get_program():
    if "nc" not in _prog_cache:
        _prog_cache["nc"] = _build_program()
    return _prog_cache["nc"]


def _device_layouts(f1q, f2q, srn1_full):
    """Host-side data marshaling into the DoubleRow SBUF layouts."""
    # f2t[p, g, kk, i, j2] = f2q[g*GW + j2, kk*256 + i*128 + p]
    f2t = np.ascontiguousarray(
        f2q.T.reshape(KK, 2, 128, NG, GW).transpose(2, 3, 0, 1, 4)
    ).reshape(128, NG * KK * 2 * GW)
    in_maps = []
    for c in range(NCORES):
        sl = slice(c * BS, (c + 1) * BS)
        f1s = f1q[sl]
        # f1t[p, kk, i, m] = f1s[m, kk*256 + i*128 + p]
        f1t = np.ascontiguousarray(
            f1s.T.reshape(KK, 2, 128, BS).transpose(2, 0, 1, 3)
        ).reshape(128, KK * 2 * BS)
        in_maps.append(
            dict(
                f1t=f1t,
                f2t=f2t,
                srn1=np.ascontiguousarray(srn1_full[sl].reshape(MT, 128).T),
            )
        )
    return in_maps


def kernel(feature1, feature2, label, _want_results=False, _trace=False):
    f1 = np.ascontiguousarray(np.asarray(feature1, dtype=np.float32))
    f2 = np.ascontiguousarray(np.asarray(feature2, dtype=np.float32))
    lab = np.asarray(label)

    f2n = f2 / np.linalg.norm(f2.astype(np.float64), axis=1, keepdims=True).astype(
        np.float32
    )
    f1q = f1.astype(ml_dtypes.float8_e4m3)
    f2q = f2n.astype(ml_dtypes.float8_e4m3)
    f1qf = f1q.astype(np.float32)
    f2qf = f2q.astype(np.float32)
    srn1_full = (
        S / np.linalg.norm(f1qf.astype(np.float64), axis=1)
    ).astype(np.float32)

    in_maps = _device_layouts(f1q, f2q, srn1_full)

    nc = _get_program()
    kw = dict(trace=True) if _trace else {}
    out = run_bass_kernel_spmd(nc, in_maps, list(range(NCORES)), **kw)
    res = out.results

    sums = np.empty(B, dtype=np.float64)
    mx = np.empty(B, dtype=np.float64)
    for c in range(NCORES):
        r = res[c]
        sl = slice(c * BS, (c + 1) * BS)
        se = np.asarray(r["se"]).astype(np.float64).reshape(128, NG, MT)
        sums[sl] = se.sum(axis=1).T.reshape(BS)
        rm = np.asarray(r["rm"]).astype(np.float64).reshape(128, MT, GW)
        mx[sl] = rm.max(axis=2).T.reshape(BS)

    # ---- host combine -------------------------------------------------
    # same-label pair list (includes the diagonal)
    order = np.argsort(lab, kind="stable")
    slab = np.asarray(lab)[order]
    _, starts, cnts = np.unique(slab, return_index=True, return_counts=True)
    I_parts, J_parts = [], []
    for st, k in zip(starts, cnts):
        rows = order[st : st + k]
        I_parts.append(np.repeat(rows, k))
        J_parts.append(np.tile(rows, k))
    I = np.concatenate(I_parts)
    J = np.concatenate(J_parts)

    # replicate the device's values at those pairs (fp32 exp of fp32 dot)
    v = np.einsum("kd,kd->k", f1qf[I], f2qf[J])
    exv = np.exp((srn1_full[I] * v).astype(np.float32))
    sum_corr = np.zeros(B, dtype=np.float64)
    np.add.at(sum_corr, I, exv.astype(np.float64))
    n_off = np.zeros(B, dtype=np.float64)
    np.add.at(n_off, I, 1.0)
    n_off -= 1.0  # off-diagonal same-label count per row
    sumoff = sums - sum_corr + n_off

    # masked row max: device max is unmasked; fix rows whose max may sit on
    # a same-label column by an exact host recompute of that row
    exb = exv.astype(ml_dtypes.bfloat16).astype(np.float64)
    same_mx = np.zeros(B, dtype=np.float64)
    np.maximum.at(same_mx, I, exb)
    collide = same_mx >= mx * (1.0 - 1e-3)
    for i in np.nonzero(collide)[0]:
        row_v = (f1qf[i][None, :] @ f2qf.T).ravel()
        exrow = (
            np.exp((srn1_full[i] * row_v).astype(np.float32))
            .astype(ml_dtypes.bfloat16)
            .astype(np.float64)
        )
        exrow[np.asarray(lab) == lab[i]] = 0.0
        mx[i] = exrow.max()

    neg = np.log(np.maximum(mx, 1.0)) / S
    f1d = f1.astype(np.float64)
    f2d = f2.astype(np.float64)
    pos = np.clip(
        (f1d * f2d).sum(1)
        / (np.linalg.norm(f1d, axis=1) * np.linalg.norm(f2d, axis=1)),
        -1.0,
        1.0,
    )
    m = EMA * np.mean(pos - neg)
    z = S * (pos - m)
    loss = np.mean(np.log(sumoff + np.exp(z)) - z)
    out_val = np.float32(loss)
    if _want_results:
        return out_val, out
    return out_val


# revision 4
# speedup vs baseline: 3.2058x; 1.0191x over previous
"""ContraFace loss kernel for 8 TRN2 NeuronCores.

Strategy: row-shard the [B, B] cosine matrix across 8 cores (1024 rows per
core), f2 replicated. The device computes, per core, the only O(B^2) work:
  acc[i, j] = f1q_i . f2qn_j        (fp8-e4m3 DoubleRow matmuls, PSUM fp32)
  ex[i, j]  = exp(s_i * acc[i, j])  (ACT, bf16 out, fp32 row-sum accum)
  rm[m]     = running elementwise max of ex, folded to 1024 wide (DVE 2x)
with s_i = S / ||f1q_i||. No masking on device: the label mask only touches
the ~B^2/4096 same-label pairs, and the host can reproduce the device's
quantized values for exactly those pairs from f1q/f2qn, so it subtracts
their exp contributions and replaces them with the exp(0)=1 the reference
requires. The host also computes pos (exact diagonal cos), the margin EMA,
and the final cross-entropy in float64; the rare rows whose unmasked argmax
lands on a same-label column are fixed by an exact host recompute of that
row.

Device notes:
  - matmuls run in MatmulPerfMode.DoubleRow: both operands fp8e4 with K
    packed two-per-partition ([128, 2, M] x [128, 2, N]), 0.5 cycles/row
  - PSUM: two [128, 2048] fp32 tiles (4 banks each) rotate PE vs ACT
  - exp reads PSUM directly; accum_out yields the row-sums for free; the
    exp pass on ACT (1 elem/cycle/partition @ 1.2 GHz) is the bottleneck
  - the first (g=0, m=0) group is processed as two 1024-wide halves, with
    the f2 panel-0 halves split across the SP and Pool DMA queues, so the
    ACT engine starts ~2us earlier
  - rm tiles are [128, 1024]: each ex tile is folded by two tensor_tensor
    max ops; final per-row max happens on the host after a 2KB/partition
    DMA per tile, alternating queues right after the g=3 updates
"""

import sys

sys.path.insert(0, "/opt/trn_rl_repo")

import numpy as np
import ml_dtypes
from contextlib import ExitStack

from concourse import bass, bacc, tile
from concourse.bass_utils import run_bass_kernel_spmd
import concourse.mybir as mybir

dt = mybir.dt
Alu = mybir.AluOpType
Act = mybir.ActivationFunctionType

B, D = 8192, 512
NCORES = 8
BS = B // NCORES          # 1024 rows per core
MT = BS // 128            # 8 m-tiles per core
GW = 2048                 # column group width (4 PSUM banks)
HGW = GW // 2
NG = B // GW              # 4 column groups
KK = D // 256             # 2 DoubleRow contraction chunks
SE_W = NG * MT + 1        # one extra accum slot for the split first group
S = 64.0
EMA = 0.99

_prog_cache = {}


def _build_program():
    nc = bacc.Bacc(None)

    f1t_d = nc.declare_dram_parameter("f1t", [128, MT * KK * 2 * 128], dt.float8e4, isOutput=False)
    f2t_d = nc.declare_dram_parameter("f2t", [128, NG * 2 * KK * 2 * HGW], dt.float8e4, isOutput=False)
    srn1_d = nc.declare_dram_parameter("srn1", [128, MT], dt.float32, isOutput=False)
    se_d = nc.declare_dram_parameter("se", [128, SE_W], dt.float32, isOutput=True)
    rm_d = nc.declare_dram_parameter("rm", [128, MT * HGW], dt.bfloat16, isOutput=True)

    f1t_v = f1t_d[:].rearrange("p (m k i c) -> p m k i c", m=MT, k=KK, i=2)
    f2t_v = f2t_d[:].rearrange("p (g h k i n) -> p g h k i n", g=NG, h=2, k=KK, i=2)
    rm_v = rm_d[:].rearrange("p (m n) -> p m n", m=MT)

    with tile.TileContext(nc) as tc, ExitStack() as ctx:
        cst = ctx.enter_context(tc.tile_pool(name="cst", bufs=1))
        exq = ctx.enter_context(tc.tile_pool(name="exq", bufs=3))
        psm = ctx.enter_context(
            tc.tile_pool(name="psm", bufs=2, space=bass.MemorySpace.PSUM)
        )

        f1t = cst.tile([128, MT, KK, 2, 128], dt.float8e4, tag="f1t")
        f2t = cst.tile([128, NG, 2, KK, 2, HGW], dt.float8e4, tag="f2t")
        srn1 = cst.tile([128, MT], dt.float32, tag="srn1")
        se = cst.tile([128, SE_W], dt.float32, tag="se")
        rms = [
            cst.tile([128, HGW], dt.bfloat16, name=f"rm{m}", tag=f"rm{m}")
            for m in range(MT)
        ]

        # input DMAs: first-group halves split across the two queues
        nc.gpsimd.dma_start(srn1[:], srn1_d[:])
        nc.gpsimd.dma_start(f1t[:, 0], f1t_v[:, 0])
        nc.sync.dma_start(f2t[:, 0, 0], f2t_v[:, 0, 0])
        nc.gpsimd.dma_start(f2t[:, 0, 1], f2t_v[:, 0, 1])
        nc.gpsimd.dma_start(f1t[:, 1:], f1t_v[:, 1:])
        nc.gpsimd.dma_start(f2t[:, 1], f2t_v[:, 1])
        nc.sync.dma_start(f2t[:, 2], f2t_v[:, 2])
        nc.sync.dma_start(f2t[:, 3], f2t_v[:, 3])

        def emit_matmuls(acc, g, m, n4s, dst_off):
            for idx, n4 in enumerate(n4s):
                h, n0 = n4 // 2, (n4 % 2) * 512
                lo = dst_off + idx * 512
                for k in range(KK):
                    nc.tensor.matmul(
                        acc[:, lo : lo + 512],
                        f1t[:, m, k, :, :],
                        f2t[:, g, h, k, :, n0 : n0 + 512],
                        start=(k == 0),
                        stop=(k == KK - 1),
                        perf_mode=mybir.MatmulPerfMode.DoubleRow,
                    )

        for g in range(NG):
            for m in range(MT):
                if g == 0 and m == 0:
                    # two 1024-wide halves so ACT starts on the first DMA half
                    for h in range(2):
                        acc = psm.tile([128, GW], dt.float32, tag="acc")
                        emit_matmuls(acc, g, m, (2 * h, 2 * h + 1), 0)
                        ex = exq.tile([128, GW], dt.bfloat16, tag="ex")
                        slot = 0 if h == 0 else NG * MT
                        nc.scalar.activation(
                            ex[:, 0:HGW],
                            acc[:, 0:HGW],
                            Act.Exp,
                            bias=0.0,
                            scale=srn1[:, 0:1],
                            accum_out=se[:, slot : slot + 1],
                        )
                        if h == 0:
                            nc.vector.tensor_copy(out=rms[0][:], in_=ex[:, 0:HGW])
                        else:
                            nc.vector.tensor_tensor(
                                out=rms[0][:], in0=rms[0][:], in1=ex[:, 0:HGW], op=Alu.max
                            )
                    continue
                acc = psm.tile([128, GW], dt.float32, tag="acc")
                emit_matmuls(acc, g, m, (0, 1, 2, 3), 0)
                ex = exq.tile([128, GW], dt.bfloat16, tag="ex")
                slot = g * MT + m
                nc.scalar.activation(
                    ex[:],
                    acc[:],
                    Act.Exp,
                    bias=0.0,
                    scale=srn1[:, m : m + 1],
                    accum_out=se[:, slot : slot + 1],
                )
                if g == 0:
                    nc.vector.tensor_copy(out=rms[m][:], in_=ex[:, 0:HGW])
                else:
                    nc.vector.tensor_tensor(
                        out=rms[m][:], in0=rms[m][:], in1=ex[:, 0:HGW], op=Alu.max
                    )
                nc.vector.tensor_tensor(
                    out=rms[m][:], in0=rms[m][:], in1=ex[:, HGW:GW], op=Alu.max
                )
                if g == NG - 1:
                    q = nc.sync if (m % 2 == 0) else nc.gpsimd
                    q.dma_start(rm_v[:, m, :], rms[m][:])

        nc.gpsimd.dma_start(se_d[:], se[:])

    if not nc.is_finalized():
        nc.finalize()
    return nc


def _get_program():
    if "nc" not in _prog_cache:
        _prog_cache["nc"] = _build_program()
    return _prog_cache["nc"]


def _device_layouts(f1q, f2q, srn1_full):
    """Host-side data marshaling into the DoubleRow SBUF layouts."""
    # f2t[p, g, h, kk, i, j1] = f2q[g*GW + h*HGW + j1, kk*256 + i*128 + p]
    f2t = np.ascontiguousarray(
        f2q.T.reshape(KK, 2, 128, NG, 2, HGW).transpose(2, 3, 4, 0, 1, 5)
    ).reshape(128, NG * 2 * KK * 2 * HGW)
    in_maps = []
    for c in range(NCORES):
        sl = slice(c * BS, (c + 1) * BS)
        f1s = f1q[sl]
        # f1t[p, m, kk, i, c] = f1s[m*128 + c, kk*256 + i*128 + p]
        f1t = np.ascontiguousarray(
            f1s.T.reshape(KK, 2, 128, MT, 128).transpose(2, 3, 0, 1, 4)
        ).reshape(128, MT * KK * 2 * 128)
        in_maps.append(
            dict(
                f1t=f1t,
                f2t=f2t,
                srn1=np.ascontiguousarray(srn1_full[sl].reshape(MT, 128).T),
            )
        )
    return in_maps


def kernel(feature1, feature2, label, _want_results=False, _trace=False):
    f1 = np.ascontiguousarray(np.asarray(feature1, dtype=np.float32))
    f2 = np.ascontiguousarray(np.asarray(feature2, dtype=np.float32))
    lab = np.asarray(label)

    f2n = f2 / np.linalg.norm(f2.astype(np.float64), axis=1, keepdims=True).astype(
        np.float32
    )
    f1q = f1.astype(ml_dtypes.float8_e4m3)
    f2q = f2n.astype(ml_dtypes.float8_e4m3)
    f1qf = f1q.astype(np.float32)
    f2qf = f2q.astype(np.float32)
    srn1_full = (
        S / np.linalg.norm(f1qf.astype(np.float64), axis=1)
    ).astype(np.float32)

    in_maps = _device_layouts(f1q, f2q, srn1_full)

    nc = _get_program()
    kw = dict(trace=True) if _trace else {}
    out = run_bass_kernel_spmd(nc, in_maps, list(range(NCORES)), **kw)
    res = out.results

    sums = np.empty(B, dtype=np.float64)
    mx = np.empty(B, dtype=np.float64)
    for c in range(NCORES):
        r = res[c]
        sl = slice(c * BS, (c + 1) * BS)
        se = np.asarray(r["se"]).astype(np.float64)
        se[:, 0] += se[:, NG * MT]
        sums[sl] = se[:, : NG * MT].reshape(128, NG, MT).sum(axis=1).T.reshape(BS)
        rm = np.asarray(r["rm"]).astype(np.float64).reshape(128, MT, HGW)
        mx[sl] = rm.max(axis=2).T.reshape(BS)

    # ---- host combine -------------------------------------------------
    # same-label pair list (includes the diagonal)
    order = np.argsort(lab, kind="stable")
    slab = np.asarray(lab)[order]
    _, starts, cnts = np.unique(slab, return_index=True, return_counts=True)
    I_parts, J_parts = [], []
    for st, k in zip(starts, cnts):
        rows = order[st : st + k]
        I_parts.append(np.repeat(rows, k))
        J_parts.append(np.tile(rows, k))
    I = np.concatenate(I_parts)
    J = np.concatenate(J_parts)

    # replicate the device's values at those pairs (fp32 exp of fp32 dot)
    v = np.einsum("kd,kd->k", f1qf[I], f2qf[J])
    exv = np.exp((srn1_full[I] * v).astype(np.float32))
    sum_corr = np.zeros(B, dtype=np.float64)
    np.add.at(sum_corr, I, exv.astype(np.float64))
    n_off = np.zeros(B, dtype=np.float64)
    np.add.at(n_off, I, 1.0)
    n_off -= 1.0  # off-diagonal same-label count per row
    sumoff = sums - sum_corr + n_off

    # masked row max: device max is unmasked; fix rows whose max may sit on
    # a same-label column by an exact host recompute of that row
    exb = exv.astype(ml_dtypes.bfloat16).astype(np.float64)
    same_mx = np.zeros(B, dtype=np.float64)
    np.maximum.at(same_mx, I, exb)
    collide = same_mx >= mx * (1.0 - 1e-3)
    for i in np.nonzero(collide)[0]:
        row_v = (f1qf[i][None, :] @ f2qf.T).ravel()
        exrow = (
            np.exp((srn1_full[i] * row_v).astype(np.float32))
            .astype(ml_dtypes.bfloat16)
            .astype(np.float64)
        )
        exrow[np.asarray(lab) == lab[i]] = 0.0
        mx[i] = exrow.max()

    neg = np.log(np.maximum(mx, 1.0)) / S
    f1d = f1.astype(np.float64)
    f2d = f2.astype(np.float64)
    pos = np.clip(
        (f1d * f2d).sum(1)
        / (np.linalg.norm(f1d, axis=1) * np.linalg.norm(f2d, axis=1)),
        -1.0,
        1.0,
    )
    m = EMA * np.mean(pos - neg)
    z = S * (pos - m)
    loss = np.mean(np.log(sumoff + np.exp(z)) - z)
    out_val = np.float32(loss)
    if _want_results:
        return out_val, out
    return out_val


# revision 10
# speedup vs baseline: 3.2160x; 1.0032x over previous
"""ContraFace loss kernel for 8 TRN2 NeuronCores.

Strategy: row-shard the [B, B] cosine matrix across 8 cores (1024 rows per
core), f2 replicated. The device computes, per core, the only O(B^2) work:
  acc[i, j] = f1q_i . f2qn_j        (fp8-e4m3 DoubleRow matmuls, PSUM fp32)
  ex[i, j]  = exp(s_i * acc[i, j])  (ACT, bf16 out, fp32 row-sum accum)
  rm[m]     = running elementwise max of ex, folded to 1024 wide (DVE 2x)
with s_i = S / ||f1q_i||. No masking on device: the label mask only touches
the ~B^2/4096 same-label pairs, and the host can reproduce the device's
quantized values for exactly those pairs from f1q/f2qn, so it subtracts
their exp contributions and replaces them with the exp(0)=1 the reference
requires. The host also computes pos (exact diagonal cos), the margin EMA,
and the final cross-entropy in float64; the rare rows whose unmasked argmax
lands on a same-label column are fixed by an exact host recompute of that
row.

Device notes:
  - matmuls run in MatmulPerfMode.DoubleRow: both operands fp8e4 with K
    packed two-per-partition ([128, 2, M] x [128, 2, N]), 0.5 cycles/row
  - PSUM: two [128, 2048] fp32 tiles (4 banks each) rotate PE vs ACT
  - exp reads PSUM directly; accum_out yields the row-sums for free; the
    exp pass on ACT (1 elem/cycle/partition @ 1.2 GHz) is the bottleneck
  - the first (g=0, m=0) group is processed as two 1024-wide halves, with
    the f2 panel-0 halves split across the SP and Pool DMA queues, so the
    ACT engine starts ~2us earlier
  - rm tiles are [128, 1024]: each ex tile is folded by two tensor_tensor
    max ops; final per-row max happens on the host after a 2KB/partition
    DMA per tile, alternating queues right after the g=3 updates
"""

import sys

sys.path.insert(0, "/opt/trn_rl_repo")

import numpy as np
import ml_dtypes
from contextlib import ExitStack

from concourse import bass, bacc, tile
from concourse.bass_utils import run_bass_kernel_spmd
import concourse.mybir as mybir

dt = mybir.dt
Alu = mybir.AluOpType
Act = mybir.ActivationFunctionType

B, D = 8192, 512
NCORES = 8
BS = B // NCORES          # 1024 rows per core
MT = BS // 128            # 8 m-tiles per core
GW = 2048                 # column group width (4 PSUM banks)
HGW = GW // 2
NG = B // GW              # 4 column groups
KK = D // 256             # 2 DoubleRow contraction chunks
SE_W = NG * MT + 3        # three extra accum slots for the split first group
S = 64.0
EMA = 0.99

_prog_cache = {}


def _build_program():
    nc = bacc.Bacc(None)

    f1t_d = nc.declare_dram_parameter("f1t", [128, MT * KK * 2 * 128], dt.float8e4, isOutput=False)
    f2t_d = nc.declare_dram_parameter("f2t", [128, NG * 2 * KK * 2 * HGW], dt.float8e4, isOutput=False)
    srn1_d = nc.declare_dram_parameter("srn1", [128, MT], dt.float32, isOutput=False)
    se_d = nc.declare_dram_parameter("se", [128, SE_W], dt.float32, isOutput=True)
    rm_d = nc.declare_dram_parameter("rm", [128, MT * HGW], dt.bfloat16, isOutput=True)
    exl_d = nc.declare_dram_parameter("exl", [128, GW], dt.bfloat16, isOutput=True)

    f1t_v = f1t_d[:].rearrange("p (m k i c) -> p m k i c", m=MT, k=KK, i=2)
    f2t_v = f2t_d[:].rearrange("p (g h k i n) -> p g h k i n", g=NG, h=2, k=KK, i=2)
    rm_v = rm_d[:].rearrange("p (m n) -> p m n", m=MT)
    exl_v = exl_d[:]

    with tile.TileContext(nc) as tc, ExitStack() as ctx:
        cst = ctx.enter_context(tc.tile_pool(name="cst", bufs=1))
        exq = ctx.enter_context(tc.tile_pool(name="exq", bufs=3))
        psm = ctx.enter_context(
            tc.tile_pool(name="psm", bufs=2, space=bass.MemorySpace.PSUM)
        )

        f1t = cst.tile([128, MT, KK, 2, 128], dt.float8e4, tag="f1t")
        f2t = cst.tile([128, NG, 2, KK, 2, HGW], dt.float8e4, tag="f2t")
        srn1 = cst.tile([128, MT], dt.float32, tag="srn1")
        se = cst.tile([128, SE_W], dt.float32, tag="se")
        warm = cst.tile([128, 1], dt.float32, tag="warm")
        warm2 = cst.tile([128, 1], dt.float32, tag="warm2")
        rms = [
            cst.tile([128, HGW], dt.bfloat16, name=f"rm{m}", tag=f"rm{m}")
            for m in range(MT)
        ]

        # pull the ACT Exp table load to t~0 via a dummy activation
        nc.vector.memset(warm[:], 0.0)
        nc.scalar.activation(warm2[:], warm[:], Act.Exp, bias=0.0, scale=1.0)

        # input DMAs; the first group's f2 panel arrives as 4 quarter-panels
        # interleaved across the SP and Pool queues so ACT can start early
        nc.sync.dma_start(f2t[:, 0, 0, :, :, 0:512], f2t_v[:, 0, 0, :, :, 0:512])
        nc.gpsimd.dma_start(f1t[:, 0], f1t_v[:, 0])
        nc.gpsimd.dma_start(srn1[:], srn1_d[:])
        nc.sync.dma_start(f2t[:, 0, 0, :, :, 512:HGW], f2t_v[:, 0, 0, :, :, 512:HGW])
        nc.gpsimd.dma_start(f2t[:, 0, 1, :, :, 0:512], f2t_v[:, 0, 1, :, :, 0:512])
        nc.gpsimd.dma_start(f2t[:, 0, 1, :, :, 512:HGW], f2t_v[:, 0, 1, :, :, 512:HGW])
        nc.sync.dma_start(f1t[:, 1:2], f1t_v[:, 1:2])
        nc.sync.dma_start(f1t[:, 2:], f1t_v[:, 2:])
        nc.gpsimd.dma_start(f2t[:, 1], f2t_v[:, 1])
        nc.sync.dma_start(f2t[:, 2], f2t_v[:, 2])
        nc.sync.dma_start(f2t[:, 3], f2t_v[:, 3])

        def emit_matmuls(acc, g, m, n4s, dst_off):
            for idx, n4 in enumerate(n4s):
                h, n0 = n4 // 2, (n4 % 2) * 512
                lo = dst_off + idx * 512
                for k in range(KK):
                    nc.tensor.matmul(
                        acc[:, lo : lo + 512],
                        f1t[:, m, k, :, :],
                        f2t[:, g, h, k, :, n0 : n0 + 512],
                        start=(k == 0),
                        stop=(k == KK - 1),
                        perf_mode=mybir.MatmulPerfMode.DoubleRow,
                    )

        for g in range(NG):
            for m in range(MT):
                if g == 0 and m == 0:
                    # four 512-wide quarters sharing one acc tile (subtile
                    # deps) so ACT starts on the first quarter-panel DMA
                    acc = psm.tile([128, GW], dt.float32, tag="acc")
                    for q in range(4):
                        emit_matmuls(acc, g, m, (q,), q * 512)
                        ex = exq.tile([128, GW], dt.bfloat16, tag="ex")
                        slot = 0 if q == 0 else NG * MT + q - 1
                        nc.scalar.activation(
                            ex[:, 0:512],
                            acc[:, q * 512 : (q + 1) * 512],
                            Act.Exp,
                            bias=0.0,
                            scale=srn1[:, 0:1],
                            accum_out=se[:, slot : slot + 1],
                        )
                        hs = slice((q % 2) * 512, (q % 2) * 512 + 512)
                        if q < 2:
                            nc.vector.tensor_copy(out=rms[0][:, hs], in_=ex[:, 0:512])
                        else:
                            nc.vector.tensor_tensor(
                                out=rms[0][:, hs], in0=rms[0][:, hs],
                                in1=ex[:, 0:512], op=Alu.max,
                            )
                    continue
                acc = psm.tile([128, GW], dt.float32, tag="acc")
                emit_matmuls(acc, g, m, (0, 1, 2, 3), 0)
                ex = exq.tile([128, GW], dt.bfloat16, tag="ex")
                slot = g * MT + m
                nc.scalar.activation(
                    ex[:],
                    acc[:],
                    Act.Exp,
                    bias=0.0,
                    scale=srn1[:, m : m + 1],
                    accum_out=se[:, slot : slot + 1],
                )
                if g == NG - 1 and m == MT - 1:
                    # final group: skip the DVE fold; ship the raw ex tile on
                    # both queues in parallel and fold it on the host
                    nc.sync.dma_start(exl_v[:, 0:HGW], ex[:, 0:HGW])
                    nc.gpsimd.dma_start(exl_v[:, HGW:GW], ex[:, HGW:GW])
                    continue
                if g == 0:
                    nc.vector.tensor_copy(out=rms[m][:], in_=ex[:, 0:HGW])
                else:
                    nc.vector.tensor_tensor(
                        out=rms[m][:], in0=rms[m][:], in1=ex[:, 0:HGW], op=Alu.max
                    )
                nc.vector.tensor_tensor(
                    out=rms[m][:], in0=rms[m][:], in1=ex[:, HGW:GW], op=Alu.max
                )
                if g == NG - 1 or (g == NG - 2 and m == MT - 1):
                    q = nc.sync if (m % 2 == 0) else nc.gpsimd
                    q.dma_start(rm_v[:, m, :], rms[m][:])

        nc.gpsimd.dma_start(se_d[:], se[:])

    if not nc.is_finalized():
        nc.finalize()
    return nc


def _get_program():
    if "nc" not in _prog_cache:
        _prog_cache["nc"] = _build_program()
    return _prog_cache["nc"]


def _device_layouts(f1q, f2q, srn1_full):
    """Host-side data marshaling into the DoubleRow SBUF layouts."""
    # f2t[p, g, h, kk, i, j1] = f2q[g*GW + h*HGW + j1, kk*256 + i*128 + p]
    f2t = np.ascontiguousarray(
        f2q.T.reshape(KK, 2, 128, NG, 2, HGW).transpose(2, 3, 4, 0, 1, 5)
    ).reshape(128, NG * 2 * KK * 2 * HGW)
    in_maps = []
    for c in range(NCORES):
        sl = slice(c * BS, (c + 1) * BS)
        f1s = f1q[sl]
        # f1t[p, m, kk, i, c] = f1s[m*128 + c, kk*256 + i*128 + p]
        f1t = np.ascontiguousarray(
            f1s.T.reshape(KK, 2, 128, MT, 128).transpose(2, 3, 0, 1, 4)
        ).reshape(128, MT * KK * 2 * 128)
        in_maps.append(
            dict(
                f1t=f1t,
                f2t=f2t,
                srn1=np.ascontiguousarray(srn1_full[sl].reshape(MT, 128).T),
            )
        )
    return in_maps


def kernel(feature1, feature2, label, _want_results=False, _trace=False):
    f1 = np.ascontiguousarray(np.asarray(feature1, dtype=np.float32))
    f2 = np.ascontiguousarray(np.asarray(feature2, dtype=np.float32))
    lab = np.asarray(label)

    f2n = f2 / np.linalg.norm(f2.astype(np.float64), axis=1, keepdims=True).astype(
        np.float32
    )
    f1q = f1.astype(ml_dtypes.float8_e4m3)
    f2q = f2n.astype(ml_dtypes.float8_e4m3)
    f1qf = f1q.astype(np.float32)
    f2qf = f2q.astype(np.float32)
    srn1_full = (
        S / np.linalg.norm(f1qf.astype(np.float64), axis=1)
    ).astype(np.float32)

    in_maps = _device_layouts(f1q, f2q, srn1_full)

    nc = _get_program()
    kw = dict(trace=True) if _trace else {}
    out = run_bass_kernel_spmd(nc, in_maps, list(range(NCORES)), **kw)
    res = out.results

    sums = np.empty(B, dtype=np.float64)
    mx = np.empty(B, dtype=np.float64)
    for c in range(NCORES):
        r = res[c]
        sl = slice(c * BS, (c + 1) * BS)
        se = np.asarray(r["se"]).astype(np.float64)
        se[:, 0] += se[:, NG * MT :].sum(axis=1)
        sums[sl] = se[:, : NG * MT].reshape(128, NG, MT).sum(axis=1).T.reshape(BS)
        rm = np.asarray(r["rm"]).astype(np.float64).reshape(128, MT, HGW)
        mxc = rm.max(axis=2)
        exl = np.asarray(r["exl"]).astype(np.float64)
        mxc[:, MT - 1] = np.maximum(mxc[:, MT - 1], exl.max(axis=1))
        mx[sl] = mxc.T.reshape(BS)

    # ---- host combine -------------------------------------------------
    # same-label pair list (includes the diagonal)
    order = np.argsort(lab, kind="stable")
    slab = np.asarray(lab)[order]
    _, starts, cnts = np.unique(slab, return_index=True, return_counts=True)
    I_parts, J_parts = [], []
    for st, k in zip(starts, cnts):
        rows = order[st : st + k]
        I_parts.append(np.repeat(rows, k))
        J_parts.append(np.tile(rows, k))
    I = np.concatenate(I_parts)
    J = np.concatenate(J_parts)

    # replicate the device's values at those pairs (fp32 exp of fp32 dot)
    v = np.einsum("kd,kd->k", f1qf[I], f2qf[J])
    exv = np.exp((srn1_full[I] * v).astype(np.float32))
    sum_corr = np.zeros(B, dtype=np.float64)
    np.add.at(sum_corr, I, exv.astype(np.float64))
    n_off = np.zeros(B, dtype=np.float64)
    np.add.at(n_off, I, 1.0)
    n_off -= 1.0  # off-diagonal same-label count per row
    sumoff = sums - sum_corr + n_off

    # masked row max: device max is unmasked; fix rows whose max may sit on
    # a same-label column by an exact host recompute of that row
    exb = exv.astype(ml_dtypes.bfloat16).astype(np.float64)
    same_mx = np.zeros(B, dtype=np.float64)
    np.maximum.at(same_mx, I, exb)
    collide = same_mx >= mx * (1.0 - 1e-3)
    for i in np.nonzero(collide)[0]:
        row_v = (f1qf[i][None, :] @ f2qf.T).ravel()
        exrow = (
            np.exp((srn1_full[i] * row_v).astype(np.float32))
            .astype(ml_dtypes.bfloat16)
            .astype(np.float64)
        )
        exrow[np.asarray(lab) == lab[i]] = 0.0
        mx[i] = exrow.max()

    neg = np.log(np.maximum(mx, 1.0)) / S
    f1d = f1.astype(np.float64)
    f2d = f2.astype(np.float64)
    pos = np.clip(
        (f1d * f2d).sum(1)
        / (np.linalg.norm(f1d, axis=1) * np.linalg.norm(f2d, axis=1)),
        -1.0,
        1.0,
    )
    m = EMA * np.mean(pos - neg)
    z = S * (pos - m)
    loss = np.mean(np.log(sumoff + np.exp(z)) - z)
    out_val = np.float32(loss)
    if _want_results:
        return out_val, out
    return out_val
